# revision 13
# baseline (speedup 1.0000x reference)
"""ConvCaps2D Trainium2 Bass kernel.

Per-core work (data-parallel over batch N=8, one batch element per NeuronCore):
  1. Grouped 3x3 conv (8 groups, 8->128 ch) as K=72 matmuls: x is replicated
     9x into SBUF with a per-replica flat shift (x9[72, 4096]), so every
     3x3 window of every output chunk is a pure access pattern over x9 —
     no per-chunk DMAs at all (9 bulk contiguous DMAs per input type).
  2. 3 rounds of dynamic routing chunk-wise over output rows (8 chunks),
     layout: partitions=(to,do)=128 for votes u/s/v, partitions=(ti,to)=128
     for routing logits b / exp(b) / couplings c.  Cross-partition
     reductions/broadcasts run on the TensorEngine as matmuls with 0/1
     selection matrices (fp32 PSUM accumulation); elementwise muls on
     VectorE/GpSimd; exp on ScalarE.  rsqrt for squash is computed with the
     bit-magic Newton iteration on the vector engines so the scalar engine
     only ever needs {Identity, Square, Exp} — a single activation table,
     no LoadActFuncSet thrash.
"""

import numpy as np
import ml_dtypes

import concourse.bacc as bacc
import concourse.mybir as mybir
import concourse.tile as tile
from contextlib import ExitStack

F32 = mybir.dt.float32
BF16 = mybir.dt.bfloat16
F32R = mybir.dt.float32r
U32 = mybir.dt.uint32
AFT = mybir.ActivationFunctionType
ALU = mybir.AluOpType

N, TI, DI, H, W = 8, 8, 8, 64, 64
TO, DO = 16, 8
HO, WO = 62, 62
NPIX = HO * WO
ROUNDS = 3
EPS = 1e-9

CHUNK_ROWS = [8] * 7 + [6]

RSQRT_MAGIC_P1 = 0x5F375A86 + 1


def _host_consts():
    c = {}
    sel_do = np.zeros((128, 16), np.float32)
    sel_do[np.arange(128), np.arange(128) // 8] = 1.0
    c["sel_do"] = sel_do
    sel_to = np.zeros((128, 8), np.float32)
    sel_to[np.arange(128), np.arange(128) // 16] = 1.0
    c["sel_to"] = sel_to
    brep_ti = np.zeros((8, 128), np.float32)
    for ti in range(8):
        brep_ti[ti, ti * 16:(ti + 1) * 16] = 1.0
    c["brep_ti"] = brep_ti
    brep_do = np.zeros((16, 128), np.float32)
    for to in range(16):
        brep_do[to, to * 8:(to + 1) * 8] = 1.0
    c["brep_do"] = brep_do
    q = np.zeros((8, 128, 128), np.float32)
    for ti in range(8):
        for to in range(16):
            q[ti, ti * 16 + to, to * 8:(to + 1) * 8] = 1.0
    c["q_all"] = q.transpose(1, 0, 2).reshape(128, 8 * 128).copy()
    sr = np.zeros((8, 128, 128), np.float32)
    for ti in range(8):
        for to in range(16):
            sr[ti, to * 8:(to + 1) * 8, ti * 16 + to] = 1.0
    c["selrep"] = sr.transpose(1, 0, 2).reshape(128, 8 * 128).copy()
    c["i128"] = np.eye(128, dtype=np.float32)
    c["i128d16"] = (np.eye(128, dtype=np.float32) / 16.0)
    return c


def build_kernel(debug=False):
    nc = bacc.Bacc("TRN2", target_bir_lowering=False, debug=False,
                   num_devices=8)
    x_d = nc.dram_tensor("x", (TI, DI, H, W), F32R, kind="ExternalInput")
    w72_d = nc.dram_tensor("w72", (72, 8 * 128), F32R, kind="ExternalInput")
    cbt_d = nc.dram_tensor("cbt", (128, 8), F32, kind="ExternalInput")
    rb_d = nc.dram_tensor("rb", (128, 1), F32, kind="ExternalInput")
    seldo_d = nc.dram_tensor("sel_do", (128, 16), BF16, kind="ExternalInput")
    selto_d = nc.dram_tensor("sel_to", (128, 8), BF16, kind="ExternalInput")
    brepti_d = nc.dram_tensor("brep_ti", (8, 128), BF16, kind="ExternalInput")
    brepdo_d = nc.dram_tensor("brep_do", (16, 128), BF16, kind="ExternalInput")
    qall_d = nc.dram_tensor("q_all", (128, 8 * 128), BF16,
                            kind="ExternalInput")
    selrep_d = nc.dram_tensor("selrep", (128, 8 * 128), BF16,
                              kind="ExternalInput")
    i128_d = nc.dram_tensor("i128", (128, 128), BF16, kind="ExternalInput")
    i128d16_d = nc.dram_tensor("i128d16", (128, 128), BF16,
                               kind="ExternalInput")
    bc_d = nc.dram_tensor("bconsts", (128, 2), F32, kind="ExternalInput")
    v_d = nc.dram_tensor("v", (TO, DO, HO, WO), F32, kind="ExternalOutput")

    xflat = x_d.ap().rearrange("t d h w -> (t d) (h w)")
    vout = v_d.ap().rearrange("t d h w -> (t d) (h w)")

    with nc.allow_low_precision(reason="bf16 routing path by design"), \
            tile.TileContext(nc) as tc, ExitStack() as ctx:
        const = ctx.enter_context(tc.tile_pool(name="const", bufs=1))

        def cload(dram, shape, name, dt=BF16):
            t = const.tile(shape, dt, name=name)
            nc.sync.dma_start(t[:], dram.ap())
            return t

        w72_sb = cload(w72_d, [72, 8 * 128], "w72_sb", F32R)
        cbt_sb = cload(cbt_d, [128, 8], "cbt_sb", F32)
        rb_sb = cload(rb_d, [128, 1], "rb_sb", F32)
        seldo_sb = cload(seldo_d, [128, 16], "seldo_sb")
        selto_sb = cload(selto_d, [128, 8], "selto_sb")
        brepti_sb = cload(brepti_d, [8, 128], "brepti_sb")
        brepdo_sb = cload(brepdo_d, [16, 128], "brepdo_sb")
        qall_sb = cload(qall_d, [128, 8 * 128], "qall_sb")
        selrep_sb = cload(selrep_d, [128, 8 * 128], "selrep_sb")
        i128_sb = cload(i128_d, [128, 128], "i128_sb")
        i128d16_sb = cload(i128d16_d, [128, 128], "i128d16_sb")
        bc_sb = cload(bc_d, [128, 2], "bc_sb", F32)
        eps_b = bc_sb[0:16, 0:1]
        one_b = bc_sb[0:16, 1:2]

        px9 = ctx.enter_context(tc.tile_pool(name="x9", bufs=1))
        puf = ctx.enter_context(tc.tile_pool(name="uf", bufs=1))
        pmid = ctx.enter_context(tc.tile_pool(name="mid", bufs=2))
        psmall = ctx.enter_context(tc.tile_pool(name="small", bufs=2))
        pprod = ctx.enter_context(tc.tile_pool(name="prod", bufs=3))
        pps = ctx.enter_context(tc.tile_pool(name="ps", bufs=1, space="PSUM"))

        mm = nc.tensor.matmul

        # ---- conv: votes u for all chunks, type-major; x9 = 9 shifted
        # replicas of x[t] so each chunk's im2col window is an AP ----
        u_full = []
        dma_engs = [nc.sync, nc.scalar, nc.gpsimd]
        for t in range(8):
            x9 = px9.tile([72, H * W], F32R, tag="x9", bufs=1,
                          name=f"x9_{t}")
            for k in range(9):
                kh, kw = k // 3, k % 3
                off = kh * W + kw
                dma_engs[k % 4].dma_start(
                    x9[k * 8:(k + 1) * 8, 0:H * W - off],
                    xflat[t * 8:(t + 1) * 8, off:H * W])
            u_t = puf.tile([128, NPIX], BF16, tag=f"u{t}", bufs=1,
                           name=f"u_{t}")
            u_full.append(u_t)
            for ci, R in enumerate(CHUNK_ROWS):
                P = R * 62
                r0 = 8 * ci
                win = x9[:, r0 * W:(r0 + R) * W].rearrange(
                    "p (r w) -> p r w", w=W)[:, :, 0:62]
                u_ps = pps.tile([128, P], F32, tag="big", bufs=2,
                                name=f"ups_{t}_{ci}")
                mm(u_ps[:].rearrange("p (r w) -> p r w", w=62),
                   w72_sb[:, t * 128:(t + 1) * 128], win,
                   start=True, stop=True)
                nc.scalar.activation(u_t[:, ci * 496:ci * 496 + P], u_ps[:],
                                     AFT.Identity, bias=cbt_sb[:, t:t + 1])

        # ---- dynamic routing, ROUND-major: all 8 chunks of round r are
        # interleaved in program order so the in-order engines always have
        # independent ready work while another chunk waits on a cross-
        # engine dependency (software pipelining; engines are in-order, so
        # chunk-major emission head-of-line blocks on every stall) ----
        e_l = [None] * 8
        b_l = [None] * 8
        NP8 = 8 * 496  # 3968: all chunks' pixels, batched squash width
        for rnd in range(ROUNDS):
            # ---- phase A: weighted sum s and n2, per chunk ----
            s_l = [None] * 8
            n2all = psmall.tile([16, NP8], F32, tag="n2all", bufs=1,
                                name=f"n2all_{rnd}")
            for ci, R in enumerate(CHUNK_ROWS):
                P = R * 62
                c0 = ci * 496
                u_tiles = [u_full[t][:, c0:c0 + P] for t in range(8)]
                s_ps = pps.tile([128, P], F32, tag="big", bufs=2,
                                name=f"sps_{ci}_{rnd}")
                if rnd == 0:
                    for t in range(8):
                        mm(s_ps[:], i128d16_sb[:], u_tiles[t],
                           start=(t == 0), stop=(t == 7))
                else:
                    e_sb = e_l[ci]
                    S_ps = pps.tile([8, P], F32, tag="small_ps", bufs=2,
                                    name=f"Sps_{ci}_{rnd}")
                    mm(S_ps[:], selto_sb[:], e_sb[:], start=True, stop=True)
                    Sr = psmall.tile([8, P], BF16, tag="Sr", bufs=3,
                                     name=f"Sr_{ci}_{rnd}")
                    nc.vector.reciprocal(Sr[:], S_ps[:])
                    rrep_ps = pps.tile([128, P], F32, tag="rep", bufs=2,
                                       name=f"rrep_{ci}_{rnd}")
                    mm(rrep_ps[:], brepti_sb[:], Sr[:], start=True, stop=True)
                    c_sb = pmid.tile([128, P], BF16, tag="c", bufs=2,
                                     name=f"c_{ci}_{rnd}")
                    nc.vector.tensor_tensor(c_sb[:], e_sb[:], rrep_ps[:],
                                            op=ALU.mult)
                    for t in range(8):
                        crep_ps = pps.tile([128, P], F32, tag="crep", bufs=2,
                                           name=f"crep_{ci}_{rnd}_{t}")
                        mm(crep_ps[:], qall_sb[:, t * 128:(t + 1) * 128],
                           c_sb[:], start=True, stop=True)
                        prod = pprod.tile([128, P], BF16, tag="prod", bufs=4,
                                          name=f"prod_{ci}_{rnd}_{t}")
                        nc.vector.tensor_tensor(prod[:], crep_ps[:],
                                                u_tiles[t], op=ALU.mult)
                        mm(s_ps[:], i128_sb[:], prod[:],
                           start=(t == 0), stop=(t == 7))
                s_sb = pmid.tile([128, P], F32, tag="s_sb", bufs=9,
                                 name=f"ssb_{ci}_{rnd}")
                nc.scalar.activation(s_sb[:], s_ps[:], AFT.Identity,
                                     bias=rb_sb[:, 0:1])
                s_l[ci] = s_sb
                s2 = pmid.tile([128, P], BF16, tag="s2", bufs=2,
                               name=f"s2_{ci}_{rnd}")
                nc.scalar.activation(s2[:], s_ps[:], AFT.Square,
                                     bias=rb_sb[:, 0:1])
                n2_ps = pps.tile([16, P], F32, tag="small_ps", bufs=2,
                                 name=f"n2ps_{ci}_{rnd}")
                mm(n2_ps[:], seldo_sb[:], s2[:], start=True, stop=True)
                nc.scalar.activation(n2all[:, c0:c0 + P], n2_ps[:],
                                     AFT.Identity)

            # ---- phase B: batched squash factor f = n2/((1+n2)sqrt(n2+eps))
            # rsqrt = exp(-0.5*ln(n2+eps)) on ScalarE, two half-batches ----
            f_all = psmall.tile([16, NP8], BF16, tag="fall", bufs=1,
                                name=f"fall_{rnd}")
            for g in range(2):
                HP = NP8 // 2
                g0 = g * HP
                n2g = n2all[:, g0:g0 + HP]
                l_g = psmall.tile([16, HP], F32, tag="tmpB", bufs=4,
                                  name=f"l_{rnd}_{g}")
                nc.scalar.activation(l_g[:], n2g, AFT.Ln, bias=eps_b)
                rs_g = psmall.tile([16, HP], F32, tag="tmpB", bufs=4,
                                   name=f"rs_{rnd}_{g}")
                nc.scalar.activation(rs_g[:], l_g[:], AFT.Exp, scale=-0.5)
                np1_g = psmall.tile([16, HP], F32, tag="tmpB", bufs=4,
                                    name=f"np1_{rnd}_{g}")
                nc.scalar.activation(np1_g[:], n2g, AFT.Identity,
                                     bias=one_b)
                qq_g = psmall.tile([16, HP], F32, tag="tmpB", bufs=4,
                                   name=f"qq_{rnd}_{g}")
                nc.vector.reciprocal(qq_g[:], np1_g[:])
                tt_g = psmall.tile([16, HP], F32, tag="tmpB", bufs=4,
                                   name=f"tt_{rnd}_{g}")
                nc.vector.scalar_tensor_tensor(tt_g[:], np1_g[:], 1.0,
                                               rs_g[:], op0=ALU.subtract,
                                               op1=ALU.mult)
                nc.vector.tensor_tensor(f_all[:, g0:g0 + HP], tt_g[:],
                                        qq_g[:], op=ALU.mult)

            # ---- phase C: v, agreement, routing logit update, per chunk ----
            for ci, R in enumerate(CHUNK_ROWS):
                P = R * 62
                c0 = ci * 496
                u_tiles = [u_full[t][:, c0:c0 + P] for t in range(8)]
                frep_ps = pps.tile([128, P], F32, tag="rep", bufs=2,
                                   name=f"frep_{ci}_{rnd}")
                mm(frep_ps[:], brepdo_sb[:], f_all[:, c0:c0 + P],
                   start=True, stop=True)
                vdt = BF16 if rnd < ROUNDS - 1 else F32
                v_sb = pmid.tile([128, P], vdt, tag="v", bufs=2,
                                 name=f"v_{ci}_{rnd}")
                nc.vector.tensor_tensor(v_sb[:], s_l[ci][:], frep_ps[:],
                                        op=ALU.mult)

                if rnd < ROUNDS - 1:
                    a_ps = pps.tile([128, P], F32, tag="rep", bufs=2,
                                    name=f"aps_{ci}_{rnd}")
                    for t in range(8):
                        prod2 = pprod.tile([128, P], BF16, tag="prod2",
                                           bufs=4,
                                           name=f"prod2_{ci}_{rnd}_{t}")
                        eng = nc.gpsimd if t in (1, 4, 7) else nc.vector
                        eng.tensor_tensor(prod2[:], u_tiles[t], v_sb[:],
                                          op=ALU.mult)
                        mm(a_ps[:], selrep_sb[:, t * 128:(t + 1) * 128],
                           prod2[:], start=(t == 0), stop=(t == 7))
                    e_sb = pmid.tile([128, P], BF16, tag="e", bufs=9,
                                     name=f"e_{ci}_{rnd}")
                    e_l[ci] = e_sb
                    if rnd == 0:
                        b_sb = pmid.tile([128, P], F32, tag="b", bufs=9,
                                         name=f"b_{ci}_{rnd}")
                        nc.scalar.activation(b_sb[:], a_ps[:], AFT.Identity)
                        nc.scalar.activation(e_sb[:], a_ps[:], AFT.Exp)
                        b_l[ci] = b_sb
                    else:
                        b1 = pmid.tile([128, P], F32, tag="b", bufs=9,
                                       name=f"b1_{ci}")
                        nc.vector.tensor_tensor(b1[:], a_ps[:], b_l[ci][:],
                                                op=ALU.add)
                        nc.scalar.activation(e_sb[:], b1[:], AFT.Exp)
                        b_l[ci] = b1
                else:
                    nc.sync.dma_start(vout[:, c0:c0 + P], v_sb[:])

    nc.compile()
    return nc


_NC_CACHE = None


def _get_nc():
    global _NC_CACHE
    if _NC_CACHE is None:
        _NC_CACHE = build_kernel()
    return _NC_CACHE


def make_in_maps(x, conv_w, conv_b, routing_bias):
    consts = _host_consts()
    bf = ml_dtypes.bfloat16
    w5 = conv_w.reshape(8, 128, 8, 3, 3)
    w72 = w5.transpose(0, 3, 4, 2, 1).reshape(8, 72, 128)
    w72_2d = np.ascontiguousarray(
        w72.transpose(1, 0, 2).reshape(72, 8 * 128)).astype(np.float32)
    cbt = np.ascontiguousarray(conv_b.reshape(8, 128).T).astype(np.float32)
    rb = routing_bias.reshape(128, 1).astype(np.float32)
    shared = dict(
        w72=w72_2d, cbt=cbt, rb=rb,
        sel_do=consts["sel_do"].astype(bf), sel_to=consts["sel_to"].astype(bf),
        brep_ti=consts["brep_ti"].astype(bf),
        brep_do=consts["brep_do"].astype(bf),
        q_all=consts["q_all"].astype(bf),
        selrep=consts["selrep"].astype(bf),
        i128=consts["i128"].astype(bf),
        i128d16=consts["i128d16"].astype(bf),
        bconsts=np.stack([np.full(128, 1e-9, np.float32),
                          np.ones(128, np.float32)], axis=1),
    )
    in_maps = []
    for n in range(N):
        m = dict(shared)
        m["x"] = np.ascontiguousarray(x[n]).astype(np.float32)
        in_maps.append(m)
    return in_maps


def kernel(x, conv_w, conv_b, routing_bias):
    from concourse.bass_utils import run_bass_kernel_spmd
    nc = _get_nc()
    in_maps = make_in_maps(x, conv_w, conv_b, routing_bias)
    res = run_bass_kernel_spmd(nc, in_maps, core_ids=list(range(N)))
    out = np.stack([r["v"] for r in res.results], axis=0)
    return out.astype(np.float32)


# revision 15
# speedup vs baseline: 1.1933x; 1.1933x over previous
"""ConvCaps2D Trainium2 Bass kernel.

Per-core work (data-parallel over batch N=8, one batch element per NeuronCore):
  1. Grouped 3x3 conv (8 groups, 8->128 ch) as K=72 matmuls: x is replicated
     9x into SBUF with a per-replica flat shift (x9[72, 4096]), so every
     3x3 window of every output chunk is a pure access pattern over x9 —
     no per-chunk DMAs at all (9 bulk contiguous DMAs per input type).
  2. 3 rounds of dynamic routing chunk-wise over output rows (8 chunks),
     layout: partitions=(to,do)=128 for votes u/s/v, partitions=(ti,to)=128
     for routing logits b / exp(b) / couplings c.  Cross-partition
     reductions/broadcasts run on the TensorEngine as matmuls with 0/1
     selection matrices (fp32 PSUM accumulation); elementwise muls on
     VectorE/GpSimd; exp on ScalarE.  rsqrt for squash is computed with the
     bit-magic Newton iteration on the vector engines so the scalar engine
     only ever needs {Identity, Square, Exp} — a single activation table,
     no LoadActFuncSet thrash.
"""

import numpy as np
import ml_dtypes

import concourse.bacc as bacc
import concourse.mybir as mybir
import concourse.tile as tile
from contextlib import ExitStack

F32 = mybir.dt.float32
BF16 = mybir.dt.bfloat16
F32R = mybir.dt.float32r
U32 = mybir.dt.uint32
AFT = mybir.ActivationFunctionType
ALU = mybir.AluOpType

N, TI, DI, H, W = 8, 8, 8, 64, 64
TO, DO = 16, 8
HO, WO = 62, 62
NPIX = HO * WO
ROUNDS = 3
EPS = 1e-9

CHUNK_ROWS = [8] * 7 + [6]

RSQRT_MAGIC_P1 = 0x5F375A86 + 1


def _host_consts():
    c = {}
    sel_do = np.zeros((128, 16), np.float32)
    sel_do[np.arange(128), np.arange(128) // 8] = 1.0
    c["sel_do"] = sel_do
    sel_to = np.zeros((128, 8), np.float32)
    sel_to[np.arange(128), np.arange(128) // 16] = 1.0
    c["sel_to"] = sel_to
    brep_ti = np.zeros((8, 128), np.float32)
    for ti in range(8):
        brep_ti[ti, ti * 16:(ti + 1) * 16] = 1.0
    c["brep_ti"] = brep_ti
    brep_do = np.zeros((16, 128), np.float32)
    for to in range(16):
        brep_do[to, to * 8:(to + 1) * 8] = 1.0
    c["brep_do"] = brep_do
    q = np.zeros((8, 128, 128), np.float32)
    for ti in range(8):
        for to in range(16):
            q[ti, ti * 16 + to, to * 8:(to + 1) * 8] = 1.0
    c["q_all"] = q.transpose(1, 0, 2).reshape(128, 8 * 128).copy()
    sr = np.zeros((8, 128, 128), np.float32)
    for ti in range(8):
        for to in range(16):
            sr[ti, to * 8:(to + 1) * 8, ti * 16 + to] = 1.0
    c["selrep"] = sr.transpose(1, 0, 2).reshape(128, 8 * 128).copy()
    c["i128"] = np.eye(128, dtype=np.float32)
    c["i128d16"] = (np.eye(128, dtype=np.float32) / 16.0)
    return c


def build_kernel(debug=False):
    nc = bacc.Bacc("TRN2", target_bir_lowering=False, debug=False,
                   num_devices=8)
    x_d = nc.dram_tensor("x", (TI, DI, H, W), F32R, kind="ExternalInput")
    w72_d = nc.dram_tensor("w72", (72, 8 * 128), F32R, kind="ExternalInput")
    cbt_d = nc.dram_tensor("cbt", (128, 8), F32, kind="ExternalInput")
    rb_d = nc.dram_tensor("rb", (128, 1), F32, kind="ExternalInput")
    seldo_d = nc.dram_tensor("sel_do", (128, 16), BF16, kind="ExternalInput")
    selto_d = nc.dram_tensor("sel_to", (128, 8), BF16, kind="ExternalInput")
    brepti_d = nc.dram_tensor("brep_ti", (8, 128), BF16, kind="ExternalInput")
    brepdo_d = nc.dram_tensor("brep_do", (16, 128), BF16, kind="ExternalInput")
    qall_d = nc.dram_tensor("q_all", (128, 8 * 128), BF16,
                            kind="ExternalInput")
    selrep_d = nc.dram_tensor("selrep", (128, 8 * 128), BF16,
                              kind="ExternalInput")
    i128_d = nc.dram_tensor("i128", (128, 128), BF16, kind="ExternalInput")
    i128d16_d = nc.dram_tensor("i128d16", (128, 128), BF16,
                               kind="ExternalInput")
    bc_d = nc.dram_tensor("bconsts", (128, 2), F32, kind="ExternalInput")
    v_d = nc.dram_tensor("v", (TO, DO, HO, WO), F32, kind="ExternalOutput")

    xflat = x_d.ap().rearrange("t d h w -> (t d) (h w)")
    vout = v_d.ap().rearrange("t d h w -> (t d) (h w)")

    with nc.allow_low_precision(reason="bf16 routing path by design"), \
            tile.TileContext(nc) as tc, ExitStack() as ctx:
        const = ctx.enter_context(tc.tile_pool(name="const", bufs=1))

        def cload(dram, shape, name, dt=BF16):
            t = const.tile(shape, dt, name=name)
            nc.sync.dma_start(t[:], dram.ap())
            return t

        w72_sb = cload(w72_d, [72, 8 * 128], "w72_sb", F32R)
        cbt_sb = cload(cbt_d, [128, 8], "cbt_sb", F32)
        rb_sb = cload(rb_d, [128, 1], "rb_sb", F32)
        seldo_sb = cload(seldo_d, [128, 16], "seldo_sb")
        selto_sb = cload(selto_d, [128, 8], "selto_sb")
        brepti_sb = cload(brepti_d, [8, 128], "brepti_sb")
        brepdo_sb = cload(brepdo_d, [16, 128], "brepdo_sb")
        qall_sb = cload(qall_d, [128, 8 * 128], "qall_sb")
        selrep_sb = cload(selrep_d, [128, 8 * 128], "selrep_sb")
        i128_sb = cload(i128_d, [128, 128], "i128_sb")
        i128d16_sb = cload(i128d16_d, [128, 128], "i128d16_sb")
        bc_sb = cload(bc_d, [128, 2], "bc_sb", F32)
        eps_b = bc_sb[0:16, 0:1]
        one_b = bc_sb[0:16, 1:2]

        px9 = ctx.enter_context(tc.tile_pool(name="x9", bufs=2))
        puf = ctx.enter_context(tc.tile_pool(name="uf", bufs=1))
        pmid = ctx.enter_context(tc.tile_pool(name="mid", bufs=2))
        psmall = ctx.enter_context(tc.tile_pool(name="small", bufs=2))
        pprod = ctx.enter_context(tc.tile_pool(name="prod", bufs=3))
        pps = ctx.enter_context(tc.tile_pool(name="ps", bufs=1, space="PSUM"))

        mm = nc.tensor.matmul

        # ---- conv: votes u for all chunks, type-major; x9 = 9 shifted
        # replicas of x[t] so each chunk's im2col window is an AP ----
        u_full = []
        dma_engs = [nc.sync, nc.scalar, nc.gpsimd]
        for t in range(8):
            x9 = px9.tile([72, H * W], F32R, tag="x9", bufs=2,
                          name=f"x9_{t}")
            for k in range(9):
                kh, kw = k // 3, k % 3
                off = kh * W + kw
                dma_engs[k % 4].dma_start(
                    x9[k * 8:(k + 1) * 8, 0:H * W - off],
                    xflat[t * 8:(t + 1) * 8, off:H * W])
            u_t = puf.tile([128, NPIX], BF16, tag=f"u{t}", bufs=1,
                           name=f"u_{t}")
            u_full.append(u_t)
            for ci, R in enumerate(CHUNK_ROWS):
                P = R * 62
                r0 = 8 * ci
                win = x9[:, r0 * W:(r0 + R) * W].rearrange(
                    "p (r w) -> p r w", w=W)[:, :, 0:62]
                u_ps = pps.tile([128, P], F32, tag="big", bufs=2,
                                name=f"ups_{t}_{ci}")
                mm(u_ps[:].rearrange("p (r w) -> p r w", w=62),
                   w72_sb[:, t * 128:(t + 1) * 128], win,
                   start=True, stop=True)
                nc.scalar.activation(u_t[:, ci * 496:ci * 496 + P], u_ps[:],
                                     AFT.Identity, bias=cbt_sb[:, t:t + 1])

        # ---- dynamic routing, ROUND-major: all 8 chunks of round r are
        # interleaved in program order so the in-order engines always have
        # independent ready work while another chunk waits on a cross-
        # engine dependency (software pipelining; engines are in-order, so
        # chunk-major emission head-of-line blocks on every stall) ----
        e_l = [None] * 8
        b_l = [None] * 8
        NP8 = 8 * 496  # 3968: all chunks' pixels, batched squash width
        for rnd in range(ROUNDS):
            # ---- phase A: weighted sum s and n2, per chunk ----
            s_l = [None] * 8
            n2all = psmall.tile([16, NP8], F32, tag="n2all", bufs=1,
                                name=f"n2all_{rnd}")
            for ci, R in enumerate(CHUNK_ROWS):
                P = R * 62
                c0 = ci * 496
                u_tiles = [u_full[t][:, c0:c0 + P] for t in range(8)]
                s_ps = pps.tile([128, P], F32, tag="big", bufs=2,
                                name=f"sps_{ci}_{rnd}")
                if rnd == 0:
                    for t in range(8):
                        mm(s_ps[:], i128d16_sb[:], u_tiles[t],
                           start=(t == 0), stop=(t == 7))
                else:
                    e_sb = e_l[ci]
                    S_ps = pps.tile([8, P], F32, tag="small_ps", bufs=2,
                                    name=f"Sps_{ci}_{rnd}")
                    mm(S_ps[:], selto_sb[:], e_sb[:], start=True, stop=True)
                    Sr = psmall.tile([8, P], BF16, tag="Sr", bufs=3,
                                     name=f"Sr_{ci}_{rnd}")
                    nc.vector.reciprocal(Sr[:], S_ps[:])
                    rrep_ps = pps.tile([128, P], F32, tag="rep", bufs=2,
                                       name=f"rrep_{ci}_{rnd}")
                    mm(rrep_ps[:], brepti_sb[:], Sr[:], start=True, stop=True)
                    c_sb = pmid.tile([128, P], BF16, tag="c", bufs=2,
                                     name=f"c_{ci}_{rnd}")
                    nc.vector.tensor_tensor(c_sb[:], e_sb[:], rrep_ps[:],
                                            op=ALU.mult)
                    for t in range(8):
                        crep_ps = pps.tile([128, P], F32, tag="crep", bufs=2,
                                           name=f"crep_{ci}_{rnd}_{t}")
                        mm(crep_ps[:], qall_sb[:, t * 128:(t + 1) * 128],
                           c_sb[:], start=True, stop=True)
                        prod = pprod.tile([128, P], BF16, tag="prod", bufs=4,
                                          name=f"prod_{ci}_{rnd}_{t}")
                        nc.vector.tensor_tensor(prod[:], crep_ps[:],
                                                u_tiles[t], op=ALU.mult)
                        mm(s_ps[:], i128_sb[:], prod[:],
                           start=(t == 0), stop=(t == 7))
                s_sb = pmid.tile([128, P], F32, tag="s_sb", bufs=9,
                                 name=f"ssb_{ci}_{rnd}")
                nc.scalar.activation(s_sb[:], s_ps[:], AFT.Identity,
                                     bias=rb_sb[:, 0:1])
                s_l[ci] = s_sb
                s2 = pmid.tile([128, P], BF16, tag="s2", bufs=2,
                               name=f"s2_{ci}_{rnd}")
                nc.scalar.activation(s2[:], s_ps[:], AFT.Square,
                                     bias=rb_sb[:, 0:1])
                n2_ps = pps.tile([16, P], F32, tag="small_ps", bufs=2,
                                 name=f"n2ps_{ci}_{rnd}")
                mm(n2_ps[:], seldo_sb[:], s2[:], start=True, stop=True)
                nc.scalar.activation(n2all[:, c0:c0 + P], n2_ps[:],
                                     AFT.Identity)

            # ---- phase B: batched squash factor f = n2/((1+n2)sqrt(n2+eps))
            # rsqrt = exp(-0.5*ln(n2+eps)) on ScalarE, two half-batches ----
            f_halves = []
            for g in range(4):
                HP = NP8 // 4
                g0 = g * HP
                f_g = psmall.tile([16, HP], BF16, tag="fall", bufs=4,
                                  name=f"fall_{rnd}_{g}")
                f_halves.append(f_g)
                n2g = n2all[:, g0:g0 + HP]
                l_g = psmall.tile([16, HP], F32, tag="tmpB", bufs=4,
                                  name=f"l_{rnd}_{g}")
                nc.scalar.activation(l_g[:], n2g, AFT.Ln, bias=eps_b)
                rs_g = psmall.tile([16, HP], F32, tag="tmpB", bufs=4,
                                   name=f"rs_{rnd}_{g}")
                nc.scalar.activation(rs_g[:], l_g[:], AFT.Exp, scale=-0.5)
                np1_g = psmall.tile([16, HP], F32, tag="tmpB", bufs=4,
                                    name=f"np1_{rnd}_{g}")
                nc.scalar.activation(np1_g[:], n2g, AFT.Identity,
                                     bias=one_b)
                qq_g = psmall.tile([16, HP], F32, tag="tmpB", bufs=4,
                                   name=f"qq_{rnd}_{g}")
                nc.vector.reciprocal(qq_g[:], np1_g[:])
                tt_g = psmall.tile([16, HP], F32, tag="tmpB", bufs=4,
                                   name=f"tt_{rnd}_{g}")
                nc.vector.scalar_tensor_tensor(tt_g[:], np1_g[:], 1.0,
                                               rs_g[:], op0=ALU.subtract,
                                               op1=ALU.mult)
                nc.vector.tensor_tensor(f_g[:], tt_g[:],
                                        qq_g[:], op=ALU.mult)

            # ---- phase C: v, agreement, routing logit update, per chunk ----
            for ci, R in enumerate(CHUNK_ROWS):
                P = R * 62
                c0 = ci * 496
                u_tiles = [u_full[t][:, c0:c0 + P] for t in range(8)]
                frep_ps = pps.tile([128, P], F32, tag="rep", bufs=2,
                                   name=f"frep_{ci}_{rnd}")
                fh = f_halves[ci // 2]
                h0 = c0 - (ci // 2) * (NP8 // 4)
                mm(frep_ps[:], brepdo_sb[:], fh[:, h0:h0 + P],
                   start=True, stop=True)
                vdt = BF16 if rnd < ROUNDS - 1 else F32
                v_sb = pmid.tile([128, P], vdt, tag="v", bufs=2,
                                 name=f"v_{ci}_{rnd}")
                nc.vector.tensor_tensor(v_sb[:], s_l[ci][:], frep_ps[:],
                                        op=ALU.mult)

                if rnd < ROUNDS - 1:
                    a_ps = pps.tile([128, P], F32, tag="rep", bufs=2,
                                    name=f"aps_{ci}_{rnd}")
                    for t in range(8):
                        prod2 = pprod.tile([128, P], BF16, tag="prod2",
                                           bufs=4,
                                           name=f"prod2_{ci}_{rnd}_{t}")
                        eng = nc.gpsimd if t in (1, 4, 7) else nc.vector
                        eng.tensor_tensor(prod2[:], u_tiles[t], v_sb[:],
                                          op=ALU.mult)
                        mm(a_ps[:], selrep_sb[:, t * 128:(t + 1) * 128],
                           prod2[:], start=(t == 0), stop=(t == 7))
                    e_sb = pmid.tile([128, P], BF16, tag="e", bufs=9,
                                     name=f"e_{ci}_{rnd}")
                    e_l[ci] = e_sb
                    if rnd == 0:
                        b_sb = pmid.tile([128, P], F32, tag="b", bufs=9,
                                         name=f"b_{ci}_{rnd}")
                        nc.scalar.activation(b_sb[:], a_ps[:], AFT.Identity)
                        nc.scalar.activation(e_sb[:], a_ps[:], AFT.Exp)
                        b_l[ci] = b_sb
                    else:
                        b1 = pmid.tile([128, P], F32, tag="b", bufs=9,
                                       name=f"b1_{ci}")
                        nc.vector.tensor_tensor(b1[:], a_ps[:], b_l[ci][:],
                                                op=ALU.add)
                        nc.scalar.activation(e_sb[:], b1[:], AFT.Exp)
                        b_l[ci] = b1
                else:
                    nc.sync.dma_start(vout[:, c0:c0 + P], v_sb[:])

    nc.compile()
    return nc


_NC_CACHE = None


def _get_nc():
    global _NC_CACHE
    if _NC_CACHE is None:
        _NC_CACHE = build_kernel()
    return _NC_CACHE


def make_in_maps(x, conv_w, conv_b, routing_bias):
    consts = _host_consts()
    bf = ml_dtypes.bfloat16
    w5 = conv_w.reshape(8, 128, 8, 3, 3)
    w72 = w5.transpose(0, 3, 4, 2, 1).reshape(8, 72, 128)
    w72_2d = np.ascontiguousarray(
        w72.transpose(1, 0, 2).reshape(72, 8 * 128)).astype(np.float32)
    cbt = np.ascontiguousarray(conv_b.reshape(8, 128).T).astype(np.float32)
    rb = routing_bias.reshape(128, 1).astype(np.float32)
    shared = dict(
        w72=w72_2d, cbt=cbt, rb=rb,
        sel_do=consts["sel_do"].astype(bf), sel_to=consts["sel_to"].astype(bf),
        brep_ti=consts["brep_ti"].astype(bf),
        brep_do=consts["brep_do"].astype(bf),
        q_all=consts["q_all"].astype(bf),
        selrep=consts["selrep"].astype(bf),
        i128=consts["i128"].astype(bf),
        i128d16=consts["i128d16"].astype(bf),
        bconsts=np.stack([np.full(128, 1e-9, np.float32),
                          np.ones(128, np.float32)], axis=1),
    )
    in_maps = []
    for n in range(N):
        m = dict(shared)
        m["x"] = np.ascontiguousarray(x[n]).astype(np.float32)
        in_maps.append(m)
    return in_maps


def kernel(x, conv_w, conv_b, routing_bias):
    from concourse.bass_utils import run_bass_kernel_spmd
    nc = _get_nc()
    in_maps = make_in_maps(x, conv_w, conv_b, routing_bias)
    res = run_bass_kernel_spmd(nc, in_maps, core_ids=list(range(N)))
    out = np.stack([r["v"] for r in res.results], axis=0)
    return out.astype(np.float32)
